# revision 8
# baseline (speedup 1.0000x reference)
# Trainium2 Bass kernel for CrossScaleFreqAttention.
#
# Math (per batch b):
#   tokens[l, n, c] = mean over the 8x8 window of {target, 4 neighbors}[l, c]
#   proj = tokens @ proj_w + proj_b ; q/k/v linear ; softmax over n (5)
#   delta[l, c] = (attn-weighted v) @ out_w + out_b
#   out = target_win + delta broadcast over the window
#
# Sharding: data-parallel over B=8 -> one batch element per NeuronCore,
# weights replicated, no cross-core communication.
#
# Per-core structure (memory-bound: 80 MiB in + 16 MiB out per core;
# the 16 SDMA engines sustain ~390 GB/s when busy => ~266 us of DMA
# work is the floor, so the schedule aims for zero DMA idle):
#   L=1024 in 8 chunks of 128 SBUF partitions.
#   - Queue split so no load ever waits behind a compute-dependent
#     store: neighbor loads (f32 -> bf16 in-DMA) on the gpsimd SWDGE
#     ring; target loads alone on the sync HWDGE queue (keeping them
#     off the SWDGE ring also keeps descriptor-ring traffic off the
#     SBUF ports the TensorE reads its moving operand through); inline
#     stores for the early chunks on the scalar HWDGE queue behind the
#     (tiny) weight loads.
#   - Neighbor pooling on the TensorEngine per neighbor k (8
#     accumulating matmuls with a stationary bf16 identity, one
#     16B-cacheline w-octet innermost), so pooling of neighbor k only
#     waits for neighbor k's own DMA. Target pooling on the VectorE
#     (its tile must stay f32 for the exact in-place final add).
#   - Token/attention chain in bf16 (fp32 PSUM accumulate everywhere;
#     delta is ~0.1% of the output, so bf16 rounding is ~1e-6 there).
#   - Stores for chunks 3-6 are issued at the END of the SWDGE ring,
#     after every neighbor load, so the DMA engines drain 8 MiB of
#     held stores exactly while chunk 7's serial compute chain runs.
#     All broadcast-adds run on the VectorE; the add for chunk j is
#     emitted with chunk j+2, early enough that held stores are never
#     gated on the Vector queue at the tail.

import math
import os

import numpy as np

B, L, C, W2 = 8, 1024, 64, 64
K, NTOK, D = 4, 5, 32
LCHUNK = 128
NCHUNK = L // LCHUNK
HALF = 64  # l-positions per half-chunk (320 = HALF*NTOK columns <= 512 PSUM)
NCORES = 8
DEFER = (3, 4, 5, 6)  # chunks whose stores are held to the end of the run

LAST_RESULTS = None  # BassKernelResults of the most recent run (for test.py)


def _build():
    from contextlib import ExitStack

    import concourse.bacc as bacc
    import concourse.mybir as mybir
    import concourse.tile as tile

    f32 = mybir.dt.float32
    bf16 = mybir.dt.bfloat16
    AX = mybir.AxisListType.X
    EXP = mybir.ActivationFunctionType.Exp

    nc = bacc.Bacc(
        "TRN2",
        target_bir_lowering=False,
        debug=False,
        num_devices=NCORES,
    )

    def din(name, shape, dt=f32):
        return nc.dram_tensor(name, shape, dt, kind="ExternalInput").ap()

    tgt = din("tgt", [L, C * W2])
    nbr = din("nbr", [K, L, C * W2])
    ident = din("ident", [128, 128], bf16)
    pw = din("pw", [C, D], bf16)  # pre-scaled by 1/64 (window mean) on host
    pb = din("pb", [D])
    qw = din("qw", [D, D], bf16)  # pre-scaled by 1/sqrt(D) on host
    qb = din("qb", [D])           # pre-scaled by 1/sqrt(D) on host
    kw = din("kw", [D, D], bf16)
    kb = din("kb", [D])
    vw = din("vw", [D, D], bf16)
    vb = din("vb", [D])
    ow = din("ow", [D, C], bf16)
    ob = din("ob", [C])
    y = nc.dram_tensor("y", [L, C * W2], f32, kind="ExternalOutput").ap()

    with (
        tile.TileContext(nc) as tc,
        ExitStack() as ctx,
        nc.allow_low_precision(reason="bf16 attention path; output add stays f32"),
    ):
        const = ctx.enter_context(tc.tile_pool(name="const", bufs=1))
        targp = ctx.enter_context(tc.tile_pool(name="targ", bufs=6))
        nbrp = ctx.enter_context(tc.tile_pool(name="nbr", bufs=2))
        tokp = ctx.enter_context(tc.tile_pool(name="tok", bufs=2))
        smallp = ctx.enter_context(tc.tile_pool(name="small", bufs=2))
        deltap = ctx.enter_context(tc.tile_pool(name="delta", bufs=8))
        ps_tok = ctx.enter_context(tc.tile_pool(name="ps_tok", bufs=1, space="PSUM"))
        ps_tt = ctx.enter_context(tc.tile_pool(name="ps_tt", bufs=1, space="PSUM"))
        ps_sm = ctx.enter_context(tc.tile_pool(name="ps_sm", bufs=3, space="PSUM"))

        # Weights ride the scalar HWDGE queue: the load/store queues stay
        # untouched so the first big load issues immediately.
        ident_s = const.tile([128, 128], bf16)
        nc.scalar.dma_start(out=ident_s[:], in_=ident)
        pw_s = const.tile([C, D], bf16)
        nc.scalar.dma_start(out=pw_s[:], in_=pw)
        qw_s = const.tile([D, D], bf16)
        nc.scalar.dma_start(out=qw_s[:], in_=qw)
        kw_s = const.tile([D, D], bf16)
        nc.scalar.dma_start(out=kw_s[:], in_=kw)
        vw_s = const.tile([D, D], bf16)
        nc.scalar.dma_start(out=vw_s[:], in_=vw)
        ow_s = const.tile([D, C], bf16)
        nc.scalar.dma_start(out=ow_s[:], in_=ow)
        pb_s = const.tile([D, 1], f32)
        nc.scalar.dma_start(out=pb_s[:], in_=pb.unsqueeze(1))
        qb_s = const.tile([D, 1], f32)
        nc.scalar.dma_start(out=qb_s[:], in_=qb.unsqueeze(1))
        kb_s = const.tile([D, 1], f32)
        nc.scalar.dma_start(out=kb_s[:], in_=kb.unsqueeze(1))
        vb_s = const.tile([D, 1], f32)
        nc.scalar.dma_start(out=vb_s[:], in_=vb.unsqueeze(1))
        ob_s = const.tile([C, 1], f32)
        nc.scalar.dma_start(out=ob_s[:], in_=ob.unsqueeze(1))
        ones_d = const.tile([D, 1], bf16)
        nc.vector.memset(ones_d[:], 1.0)
        ones_1 = const.tile([1, D], bf16)
        nc.vector.memset(ones_1[:], 1.0)

        targs = {}   # chunk -> resident f32 target tile (until stored)
        deltas = {}  # chunk -> [LCHUNK, C] delta (SBUF copy or PSUM)

        def emit_add(i, h):
            cs = slice(h * (C // 2), (h + 1) * (C // 2))
            nc.vector.tensor_add(
                targs[i][:, cs],
                targs[i][:, cs],
                deltas[i][:, cs].unsqueeze(2).to_broadcast([LCHUNK, C // 2, W2]),
            )

        def emit_store(i, h, engine):
            cs = slice(h * (C // 2), (h + 1) * (C // 2))
            yv = y[i * LCHUNK : (i + 1) * LCHUNK].rearrange(
                "l (c w) -> l c w", w=W2
            )
            engine.dma_start(out=yv[:, cs], in_=targs[i][:, cs])

        for i in range(NCHUNK):
            l0 = i * LCHUNK

            # Deferred broadcast-adds for held chunk j = i - 2: its
            # delta has been ready for a whole chunk, so these never
            # stall the in-order Vector queue, and they complete long
            # before the held stores (end of the SWDGE ring) need them.
            for j in DEFER:
                if j == i - 2 or (i == NCHUNK - 1 and j == i - 1):
                    emit_add(j, 0)
                    emit_add(j, 1)

            # ---- loads: target f32 on the sync HWDGE queue (loads
            # only, so it is never blocked), neighbors (f32 -> bf16
            # in-DMA) on the SWDGE FIFO ring.
            targ = targp.tile([LCHUNK, C, W2], f32, tag="targ")
            targs[i] = targ
            nc.sync.dma_start(
                out=targ[:],
                in_=tgt[l0 : l0 + LCHUNK].rearrange("l (c w) -> l c w", w=W2),
            )
            nbig = nbrp.tile([LCHUNK, K, C, W2], bf16, tag="nbig")
            for k in range(K):
                nc.gpsimd.dma_start(
                    out=nbig[:, k],
                    in_=nbr[k, l0 : l0 + LCHUNK].rearrange("l (c w) -> l c w", w=W2),
                )

            # ---- window pooling ----
            # Target on VectorE (tile stays f32); each neighbor k on the
            # TensorEngine as soon as its own DMA lands: 8 accumulating
            # matmuls keep one 16B bf16 cacheline (8 w-elems) innermost,
            # then one VectorE reduce folds the 8 w-slots.
            tok_t = tokp.tile([LCHUNK, C], bf16, tag="tok_t")
            tok_n = tokp.tile([LCHUNK, K * C], bf16, tag="tok_n")
            ptok8 = ps_tok.tile([LCHUNK, K, C, 8], f32, tag="ptok")
            nc.vector.reduce_sum(tok_t[:], targ[:], axis=AX)
            for k in range(K):
                for wo in range(8):
                    nc.tensor.matmul(
                        ptok8[:, k],
                        lhsT=ident_s[:],
                        rhs=nbig[:, k, :, 8 * wo : 8 * (wo + 1)],
                        start=(wo == 0),
                        stop=(wo == 7),
                    )
                nc.vector.reduce_sum(
                    tok_n[:, k * C : (k + 1) * C], ptok8[:, k], axis=AX
                )

            # ---- transpose tokens to [c, (l,n)] (l-major columns) ----
            tokT = tokp.tile([C, LCHUNK * NTOK], bf16, tag="tokT")
            tokT_ln = tokT.rearrange("c (l n) -> c l n", n=NTOK)
            for n in range(NTOK):
                ttp = ps_tt.tile([C, LCHUNK], bf16, tag="ttp")
                src_n = tok_t[:] if n == 0 else tok_n[:, (n - 1) * C : n * C]
                nc.tensor.transpose(ttp[:], src_n, ident_s[:])
                nc.scalar.copy(tokT_ln[:, :, n], ttp[:])

            fusedT = smallp.tile([D, LCHUNK], bf16, tag="fusedT")
            exps = smallp.tile([1, LCHUNK * NTOK], bf16, tag="exps")
            projs2 = []

            for h in range(2):
                cols = slice(h * HALF * NTOK, (h + 1) * HALF * NTOK)

                # proj = tokens @ pw + pb   -> [D, 320] (d on partitions)
                pproj = ps_sm.tile([D, HALF * NTOK], f32, tag="sm")
                nc.tensor.matmul(pproj[:], lhsT=pw_s[:], rhs=tokT[:, cols])
                projs = smallp.tile([D, HALF * NTOK], bf16, tag="projs")
                nc.scalar.add(projs[:], pproj[:], pb_s[:])

                # k / v over all tokens, q over token 0 only
                pk = ps_sm.tile([D, HALF * NTOK], f32, tag="sm")
                nc.tensor.matmul(pk[:], lhsT=kw_s[:], rhs=projs[:])
                ks = smallp.tile([D, HALF * NTOK], bf16, tag="ks")
                nc.scalar.add(ks[:], pk[:], kb_s[:])

                pv = ps_sm.tile([D, HALF * NTOK], f32, tag="sm")
                nc.tensor.matmul(pv[:], lhsT=vw_s[:], rhs=projs[:])
                vs = smallp.tile([D, HALF * NTOK], bf16, tag="vs")
                nc.scalar.add(vs[:], pv[:], vb_s[:])

                pq = ps_sm.tile([D, HALF], f32, tag="sm")
                nc.tensor.matmul(
                    pq[:],
                    lhsT=qw_s[:],
                    rhs=projs.rearrange("d (l n) -> d l n", n=NTOK)[:, :, 0],
                )
                qs = smallp.tile([D, HALF], bf16, tag="qs")
                nc.scalar.add(qs[:], pq[:], qb_s[:])

                # scores[l, n] = sum_d q[d, l] * k[d, (l,n)]
                qk = smallp.tile([D, HALF * NTOK], bf16, tag="qk")
                nc.vector.tensor_mul(
                    qk.rearrange("d (l n) -> d l n", n=NTOK),
                    ks.rearrange("d (l n) -> d l n", n=NTOK),
                    qs.unsqueeze(2).to_broadcast([D, HALF, NTOK]),
                )
                psc = ps_sm.tile([1, HALF * NTOK], f32, tag="sm")
                nc.tensor.matmul(psc[:], lhsT=ones_d[:], rhs=qk[:])
                # scores are O(1e-2): exp without max-shift is exact enough
                nc.scalar.activation(exps[:, cols], psc[:], EXP)
                projs2.append(vs)

            # softmax denominator for the whole chunk at once
            den = smallp.tile([1, LCHUNK], f32, tag="den")
            nc.vector.reduce_sum(
                den[:], exps.rearrange("p (l n) -> p l n", n=NTOK), axis=AX
            )
            rden = smallp.tile([1, LCHUNK], f32, tag="rden")
            nc.vector.reciprocal(rden[:], den[:])
            attn = smallp.tile([1, LCHUNK * NTOK], bf16, tag="attn")
            nc.vector.tensor_mul(
                attn.rearrange("p (l n) -> p l n", n=NTOK),
                exps.rearrange("p (l n) -> p l n", n=NTOK),
                rden.unsqueeze(2).to_broadcast([1, LCHUNK, NTOK]),
            )

            for h in range(2):
                cols = slice(h * HALF * NTOK, (h + 1) * HALF * NTOK)
                # broadcast attn over d, weight v, reduce over n
                pab = ps_sm.tile([D, HALF * NTOK], f32, tag="sm")
                nc.tensor.matmul(pab[:], lhsT=ones_1[:], rhs=attn[:, cols])
                av = smallp.tile([D, HALF * NTOK], bf16, tag="av")
                nc.vector.tensor_mul(av[:], projs2[h][:], pab[:])
                nc.vector.reduce_sum(
                    fusedT[:, h * HALF : (h + 1) * HALF],
                    av.rearrange("d (l n) -> d l n", n=NTOK),
                    axis=AX,
                )

            # delta = fused @ ow + ob  -> [c, l], then transpose to [l, c]
            pdelta = ps_sm.tile([C, LCHUNK], f32, tag="sm")
            nc.tensor.matmul(pdelta[:], lhsT=ow_s[:], rhs=fusedT[:])
            deltaT = smallp.tile([C, LCHUNK], bf16, tag="deltaT")
            nc.scalar.add(deltaT[:], pdelta[:], ob_s[:])
            pdT = ps_sm.tile([LCHUNK, C], bf16, tag="sm")
            nc.tensor.transpose(pdT[:], deltaT[:], ident_s[:C, :C])

            if i in DEFER:
                # held chunk: the delta must outlive the ps_sm rotation,
                # so park it in SBUF; add + store are emitted later
                deltaS = deltap.tile([LCHUNK, C], bf16, tag="deltaS")
                nc.scalar.copy(deltaS[:], pdT[:])
                deltas[i] = deltaS
            else:
                deltas[i] = pdT
                # inline: add + store halves pipeline VectorE vs DMA;
                # chunk 7's stores ride the (empty) sync queue
                st_eng = nc.sync if i == NCHUNK - 1 else nc.scalar
                for h in range(2):
                    emit_add(i, h)
                    emit_store(i, h, st_eng)

        # Held stores: issued at the END of the SWDGE ring, after every
        # neighbor load, so the DMA engines finish all input traffic
        # first and drain these 8 MiB while chunk 7's compute chain
        # completes.
        for j in DEFER:
            emit_store(j, 0, nc.gpsimd)
            emit_store(j, 1, nc.gpsimd)

    nc.compile()
    return nc


def kernel(
    target_win,
    neighbor_wins,
    proj_w,
    proj_b,
    q_w,
    q_b,
    k_w,
    k_b,
    v_w,
    v_b,
    out_w,
    out_b,
):
    global LAST_RESULTS
    import ml_dtypes

    from concourse.bass_utils import run_bass_kernel_spmd

    f = np.float32
    bf = ml_dtypes.bfloat16
    target_win = np.ascontiguousarray(np.asarray(target_win, f))
    neighbor_wins = np.ascontiguousarray(np.asarray(neighbor_wins, f))
    # Fold the window-mean (1/64) into proj_w and the 1/sqrt(D) score
    # scale into q_w/q_b (linear ops commute with these scalings).
    pw = (np.asarray(proj_w, f) / float(W2)).astype(bf)
    sc = 1.0 / math.sqrt(D)
    qw = (np.asarray(q_w, f) * sc).astype(bf)
    qb = np.asarray(q_b, f) * sc
    shared = {
        "ident": np.eye(128, dtype=bf),
        "pw": pw,
        "pb": np.asarray(proj_b, f),
        "qw": qw,
        "qb": qb,
        "kw": np.asarray(k_w, f).astype(bf),
        "kb": np.asarray(k_b, f),
        "vw": np.asarray(v_w, f).astype(bf),
        "vb": np.asarray(v_b, f),
        "ow": np.asarray(out_w, f).astype(bf),
        "ob": np.asarray(out_b, f),
    }
    in_maps = []
    for b in range(NCORES):
        in_maps.append(
            {
                "tgt": target_win[b].reshape(L, C * W2),
                "nbr": np.ascontiguousarray(
                    neighbor_wins[:, b].reshape(K, L, C * W2)
                ),
                **shared,
            }
        )

    nc = _build()
    res = run_bass_kernel_spmd(
        nc,
        in_maps,
        list(range(NCORES)),
        trace=bool(os.environ.get("KERNEL_PROFILE")),
    )
    LAST_RESULTS = res
    out = np.stack(
        [res.results[b]["y"].reshape(L, C, 8, 8) for b in range(NCORES)]
    )
    return out.astype(np.float32, copy=False)


# revision 9
# speedup vs baseline: 1.1880x; 1.1880x over previous
# Trainium2 Bass kernel for CrossScaleFreqAttention.
#
# Math (per batch b):
#   tokens[l, n, c] = mean over the 8x8 window of {target, 4 neighbors}[l, c]
#   proj = tokens @ proj_w + proj_b ; q/k/v linear ; softmax over n (5)
#   delta[l, c] = (attn-weighted v) @ out_w + out_b
#   out = target_win + delta broadcast over the window
#
# Sharding: data-parallel over B=8 -> one batch element per NeuronCore,
# weights replicated, no cross-core communication.
#
# Per-core structure (memory-bound: 80 MiB in + 16 MiB out per core;
# the 16 SDMA engines sustain ~390 GB/s when busy => ~260 us of DMA
# work is the floor, so the schedule aims for zero DMA idle):
#   L=1024 in 8 chunks of 128 SBUF partitions, emitted SOFTWARE-
#   PIPELINED in two stages per chunk:
#     A(i): loads + window pooling + token transpose
#     B(i): attention chain + broadcast-add + store
#   as A0 A1 B0 A2 B1 ... A7 B6 B7. Every engine's in-order queue then
#   holds chunk i+1's early ops BEFORE chunk i's late ops, so at the
#   tail chunk 7's chain overlaps chunk 6's instead of queuing behind
#   it (the chains share the scalar/vector queues).
#   - Queues: neighbor loads (f32 -> bf16 in-DMA) on the gpsimd SWDGE
#     ring; target loads + early stores + chunk 7's store on the sync
#     HWDGE queue (orders: T0 T1 st0 T2 st1 T3 st2 T4..T7 st7 — each
#     store's add is ready ~20 us before the following load must
#     issue); weights on the scalar HWDGE queue.
#   - Stores for chunks 3-6 are issued at the END of the SWDGE ring,
#     behind every neighbor load, so the DMA engines drain 8 MiB of
#     held stores exactly while chunk 7's compute chain runs.
#   - Neighbor pooling on the TensorEngine per neighbor k (8
#     accumulating matmuls with a stationary bf16 identity, one
#     16B-cacheline w-octet innermost), so chunk 7's pooling trails
#     only the last neighbor's DMA, not all four. Target pooling on
#     the VectorE (its tile must stay f32 for the exact in-place add).
#   - Token/attention chain in bf16 (fp32 PSUM accumulate everywhere;
#     delta is ~0.1% of the output, so bf16 rounding is ~1e-6 there).

import math
import os

import numpy as np

B, L, C, W2 = 8, 1024, 64, 64
K, NTOK, D = 4, 5, 32
LCHUNK = 128
NCHUNK = L // LCHUNK
HALF = 64  # l-positions per half-chunk (320 = HALF*NTOK columns <= 512 PSUM)
NCORES = 8
DEFER = (3, 4, 5, 6)  # chunks whose stores are held to the end of the run

LAST_RESULTS = None  # BassKernelResults of the most recent run (for test.py)


def _build():
    from contextlib import ExitStack

    import concourse.bacc as bacc
    import concourse.mybir as mybir
    import concourse.tile as tile

    f32 = mybir.dt.float32
    bf16 = mybir.dt.bfloat16
    AX = mybir.AxisListType.X
    EXP = mybir.ActivationFunctionType.Exp

    nc = bacc.Bacc(
        "TRN2",
        target_bir_lowering=False,
        debug=False,
        num_devices=NCORES,
    )

    def din(name, shape, dt=f32):
        return nc.dram_tensor(name, shape, dt, kind="ExternalInput").ap()

    tgt = din("tgt", [L, C * W2])
    nbr = din("nbr", [K, L, C * W2])
    ident = din("ident", [128, 128], bf16)
    pw = din("pw", [C, D], bf16)  # pre-scaled by 1/64 (window mean) on host
    pb = din("pb", [D])
    qw = din("qw", [D, D], bf16)  # pre-scaled by 1/sqrt(D) on host
    qb = din("qb", [D])           # pre-scaled by 1/sqrt(D) on host
    kw = din("kw", [D, D], bf16)
    kb = din("kb", [D])
    vw = din("vw", [D, D], bf16)
    vb = din("vb", [D])
    ow = din("ow", [D, C], bf16)
    ob = din("ob", [C])
    y = nc.dram_tensor("y", [L, C * W2], f32, kind="ExternalOutput").ap()

    with (
        tile.TileContext(nc) as tc,
        ExitStack() as ctx,
        nc.allow_low_precision(reason="bf16 attention path; output add stays f32"),
    ):
        const = ctx.enter_context(tc.tile_pool(name="const", bufs=1))
        targp = ctx.enter_context(tc.tile_pool(name="targ", bufs=6))
        nbrp = ctx.enter_context(tc.tile_pool(name="nbr", bufs=2))
        tokp = ctx.enter_context(tc.tile_pool(name="tok", bufs=2))
        smallp = ctx.enter_context(tc.tile_pool(name="small", bufs=2))
        ps_tok = ctx.enter_context(tc.tile_pool(name="ps_tok", bufs=1, space="PSUM"))
        ps_tt = ctx.enter_context(tc.tile_pool(name="ps_tt", bufs=1, space="PSUM"))
        ps_sm = ctx.enter_context(tc.tile_pool(name="ps_sm", bufs=3, space="PSUM"))

        # Weights ride the scalar HWDGE queue: the load/store queues stay
        # untouched so the first big load issues immediately.
        ident_s = const.tile([128, 128], bf16)
        nc.scalar.dma_start(out=ident_s[:], in_=ident)
        pw_s = const.tile([C, D], bf16)
        nc.scalar.dma_start(out=pw_s[:], in_=pw)
        qw_s = const.tile([D, D], bf16)
        nc.scalar.dma_start(out=qw_s[:], in_=qw)
        kw_s = const.tile([D, D], bf16)
        nc.scalar.dma_start(out=kw_s[:], in_=kw)
        vw_s = const.tile([D, D], bf16)
        nc.scalar.dma_start(out=vw_s[:], in_=vw)
        ow_s = const.tile([D, C], bf16)
        nc.scalar.dma_start(out=ow_s[:], in_=ow)
        pb_s = const.tile([D, 1], f32)
        nc.scalar.dma_start(out=pb_s[:], in_=pb.unsqueeze(1))
        qb_s = const.tile([D, 1], f32)
        nc.scalar.dma_start(out=qb_s[:], in_=qb.unsqueeze(1))
        kb_s = const.tile([D, 1], f32)
        nc.scalar.dma_start(out=kb_s[:], in_=kb.unsqueeze(1))
        vb_s = const.tile([D, 1], f32)
        nc.scalar.dma_start(out=vb_s[:], in_=vb.unsqueeze(1))
        ob_s = const.tile([C, 1], f32)
        nc.scalar.dma_start(out=ob_s[:], in_=ob.unsqueeze(1))
        ones_d = const.tile([D, 1], bf16)
        nc.vector.memset(ones_d[:], 1.0)
        ones_1 = const.tile([1, D], bf16)
        nc.vector.memset(ones_1[:], 1.0)

        state = {}  # chunk -> (targ, tokT) carried from stage A to B

        def emit_store(i, h, engine):
            cs = slice(h * (C // 2), (h + 1) * (C // 2))
            yv = y[i * LCHUNK : (i + 1) * LCHUNK].rearrange(
                "l (c w) -> l c w", w=W2
            )
            engine.dma_start(out=yv[:, cs], in_=state[i][0][:, cs])

        def emit_A(i):
            l0 = i * LCHUNK

            # ---- loads: target f32 on the sync HWDGE queue, neighbors
            # (f32 -> bf16 in-DMA) on the SWDGE FIFO ring.
            targ = targp.tile([LCHUNK, C, W2], f32, tag="targ")
            nc.sync.dma_start(
                out=targ[:],
                in_=tgt[l0 : l0 + LCHUNK].rearrange("l (c w) -> l c w", w=W2),
            )
            nbig = nbrp.tile([LCHUNK, K, C, W2], bf16, tag="nbig")
            for k in range(K):
                nc.gpsimd.dma_start(
                    out=nbig[:, k],
                    in_=nbr[k, l0 : l0 + LCHUNK].rearrange("l (c w) -> l c w", w=W2),
                )

            # ---- window pooling ----
            tok_t = tokp.tile([LCHUNK, C], bf16, tag="tok_t")
            tok_n = tokp.tile([LCHUNK, K * C], bf16, tag="tok_n")
            ptok8 = ps_tok.tile([LCHUNK, K, 4, 16, 8], f32, tag="ptok")
            nc.vector.reduce_sum(tok_t[:], targ[:], axis=AX)
            nbig_v = nbig.rearrange("l k (cg c) w -> l k cg c w", cg=4)
            for k in range(K):
                for wo in range(8):
                    nc.tensor.matmul(
                        ptok8[:, k],
                        lhsT=ident_s[:],
                        rhs=nbig_v[:, k, :, :, 8 * wo : 8 * (wo + 1)],
                        start=(wo == 0),
                        stop=(wo == 7),
                    )
                nc.vector.reduce_sum(
                    tok_n[:, k * C : (k + 1) * C].rearrange(
                        "l (cg c) -> l cg c", cg=4
                    ),
                    ptok8[:, k],
                    axis=AX,
                )

            # ---- transpose tokens to [c, (l,n)] (l-major columns) ----
            tokT = tokp.tile([C, LCHUNK * NTOK], bf16, tag="tokT")
            tokT_ln = tokT.rearrange("c (l n) -> c l n", n=NTOK)
            for n in range(NTOK):
                ttp = ps_tt.tile([C, LCHUNK], bf16, tag="ttp")
                src_n = tok_t[:] if n == 0 else tok_n[:, (n - 1) * C : n * C]
                nc.tensor.transpose(ttp[:], src_n, ident_s[:])
                nc.scalar.copy(tokT_ln[:, :, n], ttp[:])

            state[i] = (targ, tokT)

        def emit_B(i):
            targ, tokT = state[i]

            fusedT = smallp.tile([D, LCHUNK], bf16, tag="fusedT")
            exps = smallp.tile([1, LCHUNK * NTOK], bf16, tag="exps")
            projs2 = []

            for h in range(2):
                cols = slice(h * HALF * NTOK, (h + 1) * HALF * NTOK)

                # proj = tokens @ pw + pb   -> [D, 320] (d on partitions)
                pproj = ps_sm.tile([D, HALF * NTOK], f32, tag="sm")
                nc.tensor.matmul(pproj[:], lhsT=pw_s[:], rhs=tokT[:, cols])
                projs = smallp.tile([D, HALF * NTOK], bf16, tag="projs")
                nc.scalar.add(projs[:], pproj[:], pb_s[:])

                # k / v over all tokens, q over token 0 only
                pk = ps_sm.tile([D, HALF * NTOK], f32, tag="sm")
                nc.tensor.matmul(pk[:], lhsT=kw_s[:], rhs=projs[:])
                ks = smallp.tile([D, HALF * NTOK], bf16, tag="ks")
                nc.scalar.add(ks[:], pk[:], kb_s[:])

                pv = ps_sm.tile([D, HALF * NTOK], f32, tag="sm")
                nc.tensor.matmul(pv[:], lhsT=vw_s[:], rhs=projs[:])
                vs = smallp.tile([D, HALF * NTOK], bf16, tag="vs")
                nc.scalar.add(vs[:], pv[:], vb_s[:])

                pq = ps_sm.tile([D, HALF], f32, tag="sm")
                nc.tensor.matmul(
                    pq[:],
                    lhsT=qw_s[:],
                    rhs=projs.rearrange("d (l n) -> d l n", n=NTOK)[:, :, 0],
                )
                qs = smallp.tile([D, HALF], bf16, tag="qs")
                nc.scalar.add(qs[:], pq[:], qb_s[:])

                # scores[l, n] = sum_d q[d, l] * k[d, (l,n)]
                qk = smallp.tile([D, HALF * NTOK], bf16, tag="qk")
                nc.vector.tensor_mul(
                    qk.rearrange("d (l n) -> d l n", n=NTOK),
                    ks.rearrange("d (l n) -> d l n", n=NTOK),
                    qs.unsqueeze(2).to_broadcast([D, HALF, NTOK]),
                )
                psc = ps_sm.tile([1, HALF * NTOK], f32, tag="sm")
                nc.tensor.matmul(psc[:], lhsT=ones_d[:], rhs=qk[:])
                # scores are O(1e-2): exp without max-shift is exact enough
                nc.scalar.activation(exps[:, cols], psc[:], EXP)
                projs2.append(vs)

            # softmax denominator for the whole chunk at once
            den = smallp.tile([1, LCHUNK], f32, tag="den")
            nc.vector.reduce_sum(
                den[:], exps.rearrange("p (l n) -> p l n", n=NTOK), axis=AX
            )
            rden = smallp.tile([1, LCHUNK], f32, tag="rden")
            nc.vector.reciprocal(rden[:], den[:])
            attn = smallp.tile([1, LCHUNK * NTOK], bf16, tag="attn")
            nc.vector.tensor_mul(
                attn.rearrange("p (l n) -> p l n", n=NTOK),
                exps.rearrange("p (l n) -> p l n", n=NTOK),
                rden.unsqueeze(2).to_broadcast([1, LCHUNK, NTOK]),
            )

            for h in range(2):
                cols = slice(h * HALF * NTOK, (h + 1) * HALF * NTOK)
                # broadcast attn over d, weight v, reduce over n
                pab = ps_sm.tile([D, HALF * NTOK], f32, tag="sm")
                nc.tensor.matmul(pab[:], lhsT=ones_1[:], rhs=attn[:, cols])
                av = smallp.tile([D, HALF * NTOK], bf16, tag="av")
                nc.vector.tensor_mul(av[:], projs2[h][:], pab[:])
                nc.vector.reduce_sum(
                    fusedT[:, h * HALF : (h + 1) * HALF],
                    av.rearrange("d (l n) -> d l n", n=NTOK),
                    axis=AX,
                )

            # delta = fused @ ow + ob  -> [c, l], then transpose to [l, c]
            pdelta = ps_sm.tile([C, LCHUNK], f32, tag="sm")
            nc.tensor.matmul(pdelta[:], lhsT=ow_s[:], rhs=fusedT[:])
            deltaT = smallp.tile([C, LCHUNK], bf16, tag="deltaT")
            nc.scalar.add(deltaT[:], pdelta[:], ob_s[:])
            pdT = ps_sm.tile([LCHUNK, C], bf16, tag="sm")
            nc.tensor.transpose(pdT[:], deltaT[:], ident_s[:C, :C])

            # in-place broadcast-add on the VectorE; store halves
            # pipeline against the adds (held chunks store later)
            for h in range(2):
                cs = slice(h * (C // 2), (h + 1) * (C // 2))
                nc.vector.tensor_add(
                    targ[:, cs],
                    targ[:, cs],
                    pdT[:, cs].unsqueeze(2).to_broadcast([LCHUNK, C // 2, W2]),
                )
                if i not in DEFER:
                    emit_store(i, h, nc.sync)

        emit_A(0)
        emit_A(1)
        for i in range(NCHUNK):
            emit_B(i)
            if i + 2 < NCHUNK:
                emit_A(i + 2)

        # Held stores: issued at the END of the SWDGE ring, after every
        # neighbor load, so the DMA engines finish all input traffic
        # first and drain these 8 MiB while chunk 7's compute chain
        # completes.
        for j in DEFER:
            emit_store(j, 0, nc.gpsimd)
            emit_store(j, 1, nc.gpsimd)

    nc.compile()
    return nc


def kernel(
    target_win,
    neighbor_wins,
    proj_w,
    proj_b,
    q_w,
    q_b,
    k_w,
    k_b,
    v_w,
    v_b,
    out_w,
    out_b,
):
    global LAST_RESULTS
    import ml_dtypes

    from concourse.bass_utils import run_bass_kernel_spmd

    f = np.float32
    bf = ml_dtypes.bfloat16
    target_win = np.ascontiguousarray(np.asarray(target_win, f))
    neighbor_wins = np.ascontiguousarray(np.asarray(neighbor_wins, f))
    # Fold the window-mean (1/64) into proj_w and the 1/sqrt(D) score
    # scale into q_w/q_b (linear ops commute with these scalings).
    pw = (np.asarray(proj_w, f) / float(W2)).astype(bf)
    sc = 1.0 / math.sqrt(D)
    qw = (np.asarray(q_w, f) * sc).astype(bf)
    qb = np.asarray(q_b, f) * sc
    shared = {
        "ident": np.eye(128, dtype=bf),
        "pw": pw,
        "pb": np.asarray(proj_b, f),
        "qw": qw,
        "qb": qb,
        "kw": np.asarray(k_w, f).astype(bf),
        "kb": np.asarray(k_b, f),
        "vw": np.asarray(v_w, f).astype(bf),
        "vb": np.asarray(v_b, f),
        "ow": np.asarray(out_w, f).astype(bf),
        "ob": np.asarray(out_b, f),
    }
    in_maps = []
    for b in range(NCORES):
        in_maps.append(
            {
                "tgt": target_win[b].reshape(L, C * W2),
                "nbr": np.ascontiguousarray(
                    neighbor_wins[:, b].reshape(K, L, C * W2)
                ),
                **shared,
            }
        )

    nc = _build()
    res = run_bass_kernel_spmd(
        nc,
        in_maps,
        list(range(NCORES)),
        trace=bool(os.environ.get("KERNEL_PROFILE")),
    )
    LAST_RESULTS = res
    out = np.stack(
        [res.results[b]["y"].reshape(L, C, 8, 8) for b in range(NCORES)]
    )
    return out.astype(np.float32, copy=False)
